# revision 11
# baseline (speedup 1.0000x reference)
"""Trainium2 Bass kernel for nn_MultiHeadAttention (B=2, L=4096, D=512, H=8).

Sharding: 16 (batch, head) attention slices over 8 NeuronCores — each core
handles one batch and two heads (core c -> batch c//4, heads 2*(c%4), 2*(c%4)+1).

Per-core device pipeline (all layouts chosen so the TensorE contraction dim is
on SBUF partitions and the big attention output DMAs are contiguous):
  - project qT/kT (head-dim on partitions, seq on free) and V (seq on
    partitions) from host-transposed inputs; bq/bk enter through an extra
    ones-row matmul term, the key-mask through an extra mask row appended to
    kT (dh 64->65) paired with a ones row on qT.
  - per 512-wide q block: S^T = kT.T @ qT per 128-wide k chunk (fp32r
    matmuls, two chunks per 2-bank PSUM tile), exp on ScalarE into SBUF, then
    answer_aug^T += V_aug.T @ E^T where V_aug carries a ones column so row 64
    of the accumulator is the softmax row sum.
  - reciprocals of the row sums (both orientations, via a PE transpose of the
    row-sum row and a GPSIMD partition-broadcast), normalize the answer,
    output projection partials out_pT = Wo_stack.T @ ansT into DRAM.
  - attention output: PE-transpose E^T tiles back to natural (q-partition)
    layout, fuse the softmax normalization into the PSUM->SBUF move
    (tensor_scalar with a per-partition reciprocal), pair two 512-wide strips
    and DMA via the otherwise-idle GPSIMD/SWDGE queue (4 KB lines).
Host: builds per-core transposed inputs and a single packed weight image,
sums the per-core output projection partials, and adds the bv/bo contribution
(exact because softmax rows sum to one).
"""

import os
import sys

import numpy as np

for _p in (
    "/root/.axon_site",
    "/root/.axon_site/_ro/trn_rl_repo",
    "/root/.axon_site/_ro/pypackages",
    "/opt/trn_rl_repo",
):
    if os.path.isdir(_p) and _p not in sys.path:
        sys.path.append(_p)

B, L, D, H, DH = 2, 4096, 512, 8, 64
NQB = 8      # q blocks of 512
NKC = 32     # k chunks of 128

# packed-weight column offsets: wq(5x128) wk(5x128) wv(4x128) ident wo0 wo1
WQ_OFF, WK_OFF, WV_OFF = 0, 640, 1280
ID_OFF, WO_OFF = 1792, 1920
BQ_COL, BK_COL = 2944, 2945
WP_COLS = 2946

_PROGRAM = None


def _build_program():
    import concourse.mybir as mybir
    import concourse.tile as tile
    from concourse import bacc

    dt = mybir.dt
    f32, f32r = dt.float32, dt.float32r
    AF = mybir.ActivationFunctionType
    from contextlib import ExitStack

    nc = bacc.Bacc()
    xq = nc.declare_dram_parameter("xq", [D, L], f32r, isOutput=False).ap()
    xk = nc.declare_dram_parameter("xk", [D, L], f32r, isOutput=False).ap()
    xv = nc.declare_dram_parameter("xv", [D, L], f32r, isOutput=False).ap()
    wpk_d = nc.declare_dram_parameter("wpk", [128, WP_COLS], f32r,
                                      isOutput=False).ap()
    aux = nc.declare_dram_parameter("aux", [2, L], f32r, isOutput=False).ap()
    attn = nc.declare_dram_parameter("attn", [2, L, L], f32, isOutput=True).ap()
    outp = nc.declare_dram_parameter("outp", [D, L], f32, isOutput=True).ap()

    with ExitStack() as ctx:
        tc = ctx.enter_context(tile.TileContext(nc))
        persist = ctx.enter_context(tc.tile_pool(name="persist", bufs=1))
        small = ctx.enter_context(tc.tile_pool(name="small", bufs=2))
        ansp = ctx.enter_context(tc.tile_pool(name="ansp", bufs=2))
        outsb = ctx.enter_context(tc.tile_pool(name="outsb", bufs=2))
        ppo = ctx.enter_context(tc.tile_pool(name="ppo", bufs=1, space="PSUM"))

        wpk = persist.tile([128, WP_COLS], f32r, tag="wpk")
        nc.sync.dma_start(out=wpk, in_=wpk_d)
        identr = wpk[:, ID_OFF:ID_OFF + 128]
        wq_sb = [wpk[0:(128 if ci < 4 else 1),
                     WQ_OFF + ci * 128:WQ_OFF + ci * 128 + 128]
                 for ci in range(5)]
        wk_sb = [wpk[0:(128 if ci < 4 else 1),
                     WK_OFF + ci * 128:WK_OFF + ci * 128 + 128]
                 for ci in range(5)]
        wv_sb = [wpk[0:128, WV_OFF + ci * 128:WV_OFF + ci * 128 + 128]
                 for ci in range(4)]
        wo_sbh = [wpk[0:64, WO_OFF + h * 512:WO_OFF + (h + 1) * 512]
                  for h in range(2)]

        rsf = persist.tile([128, 512], f32, tag="rsf")
        nc.vector.memset(rsf, 0.0)

        qT = [persist.tile([65, L], f32r, tag=f"qT{h}", name=f"qT{h}")
              for h in range(2)]
        kT = [persist.tile([65, L], f32r, tag=f"kT{h}", name=f"kT{h}")
              for h in range(2)]
        v2h = persist.tile([128, NKC, 130], f32r, tag="v2h")
        for h in range(2):
            nc.sync.dma_start(out=qT[h][64:65, :], in_=aux[0:1, :])
            nc.sync.dma_start(out=kT[h][64:65, :], in_=aux[1:2, :])
        nc.sync.dma_start(out=v2h[:, :, 64:65], in_=aux[0:1, :])
        nc.sync.dma_start(out=v2h[:, :, 129:130], in_=aux[0:1, :])

        # ---- projections (scoped pools; ci-outer accumulation into 8
        # psum banks so the 2 MB chunk loads double-buffer with compute) ----
        with tc.tile_pool(name="stage", bufs=4) as stage, \
             tc.tile_pool(name="pop", bufs=8, space="PSUM") as pop:
            for xap, wsb, dst in ((xq, wq_sb, qT), (xk, wk_sb, kT)):
                bc = BQ_COL if dst is qT else BK_COL
                bcol = wpk[:, bc:bc + 1]
                pss = [pop.tile([128, 512], f32, tag="pop", name="pop")
                       for _ in range(8)]
                for ci in range(4):
                    xt = stage.tile([128, L], f32r, tag="xs", name="xs")
                    nc.sync.dma_start(out=xt, in_=xap[ci * 128:(ci + 1) * 128, :])
                    for n in range(8):
                        nc.tensor.matmul(pss[n], lhsT=wsb[ci],
                                         rhs=xt[:, n * 512:(n + 1) * 512],
                                         start=(ci == 0), stop=(ci == 3))
                for n in range(8):
                    ns = slice(n * 512, (n + 1) * 512)
                    nc.scalar.add(dst[0][0:64, ns], pss[n][0:64, :],
                                  bcol[0:64, :].bitcast(f32))
                    nc.scalar.add(dst[1][0:64, ns], pss[n][64:128, :],
                                  bcol[64:128, :].bitcast(f32))

            xtv = []
            for ci in range(4):
                xt = stage.tile([128, L], f32r, tag="xs", name="xs")
                nc.sync.dma_start(out=xt, in_=xv[ci * 128:(ci + 1) * 128, :])
                xtv.append(xt)
            for n4 in range(8):
                psv = pop.tile([128, 512], f32, tag="pop", name="pop")
                for nn in range(4):
                    cs = slice(n4 * 512 + nn * 128, n4 * 512 + (nn + 1) * 128)
                    # strip-at-a-time: a start=True clears the whole bank's
                    # has_written bits, so each strip's 4-matmul group must
                    # finish before the next strip begins
                    for ci in range(4):
                        nc.tensor.matmul(psv[:, nn * 128:(nn + 1) * 128],
                                         lhsT=xtv[ci][:, cs], rhs=wv_sb[ci],
                                         start=(ci == 0), stop=(ci == 3))
                for nn in range(4):
                    kc = n4 * 4 + nn
                    nc.scalar.copy(v2h[:, kc, 0:64],
                                   psv[:, nn * 128:nn * 128 + 64])
                    nc.scalar.copy(v2h[:, kc, 65:129],
                                   psv[:, nn * 128 + 64:(nn + 1) * 128])

        # ---- main attention loop ----
        ppo = ctx.enter_context(tc.tile_pool(name="ppo", bufs=1, space="PSUM"))
        pps = ctx.enter_context(tc.tile_pool(name="pps", bufs=2, space="PSUM"))
        ppa = ctx.enter_context(tc.tile_pool(name="ppa", bufs=1, space="PSUM"))
        ppt = ctx.enter_context(tc.tile_pool(name="ppt", bufs=2, space="PSUM"))
        etp = ctx.enter_context(tc.tile_pool(name="etp", bufs=17))
        pnp = ctx.enter_context(tc.tile_pool(name="pnp", bufs=6))
        for qb in range(NQB):
            qs = slice(qb * 512, (qb + 1) * 512)
            ansT = [None, None]
            for h in range(2):
                ps_a = ppa.tile([128, 512], f32, tag="pa", name="pa")
                ets = []
                for kc2 in range(NKC // 2):
                    ps_s = pps.tile([128, 1024], f32, tag="ps", name="ps")
                    for hf in range(2):
                        kc = 2 * kc2 + hf
                        nc.tensor.matmul(ps_s[:, hf * 512:(hf + 1) * 512],
                                         lhsT=kT[h][:, kc * 128:(kc + 1) * 128],
                                         rhs=qT[h][:, qs], start=True, stop=True)
                    et = etp.tile([128, 1024], f32r, tag="et", name="et")
                    nc.scalar.activation(et, ps_s, AF.Exp)
                    for hf in range(2):
                        kc = 2 * kc2 + hf
                        nc.tensor.matmul(ps_a[0:65, :],
                                         lhsT=v2h[:, kc, 65 * h:65 * h + 65],
                                         rhs=et[:, hf * 512:(hf + 1) * 512],
                                         start=(kc == 0), stop=(kc == NKC - 1))
                        ets.append(et[:, hf * 512:(hf + 1) * 512])
                # row sums -> reciprocals in both orientations
                nc.scalar.copy(rsf[0:1, :], ps_a[64:65, :])
                ps_r = ppt.tile([128, 512], f32, tag="pt", name="pt")
                for j in range(4):
                    nc.tensor.transpose(ps_r[:, j * 128:(j + 1) * 128],
                                        rsf[:, j * 128:(j + 1) * 128],
                                        identr.bitcast(f32))
                recn = small.tile([128, 4], f32, tag="recn")
                nc.vector.reciprocal(
                    recn, ps_r.rearrange("p (j i) -> p j i", i=128)[:, :, 0])
                rr = small.tile([1, 512], f32, tag="rr")
                nc.vector.reciprocal(rr, rsf[0:1, :])
                rbc = small.tile([64, 512], f32, tag="rbc")
                nc.gpsimd.partition_broadcast(rbc, rr)
                at = ansp.tile([64, 512], f32r, tag=f"ans{h}", name=f"ans{h}")
                nc.vector.tensor_mul(at, ps_a[0:64, :], rbc.bitcast(f32r))
                ansT[h] = at
                # attention out: transpose + normalized PSUM->SBUF, DMA pairs
                pn = [None] * 4
                for g in range(8):
                    for j in range(4):
                        ps_t = ppt.tile([128, 512], f32r, tag="pt", name="pt")
                        for t in range(4):
                            nc.tensor.transpose(
                                ps_t[:, t * 128:(t + 1) * 128],
                                ets[g * 4 + t][:, j * 128:(j + 1) * 128],
                                identr)
                        if g % 2 == 0:
                            pn[j] = pnp.tile([128, 1024], f32, tag="pn",
                                             name="pn")
                        nc.vector.tensor_scalar_mul(
                            pn[j][:, (g % 2) * 512:(g % 2 + 1) * 512],
                            ps_t, recn[:, j:j + 1])
                        if g % 2 == 1:
                            nc.gpsimd.dma_start(
                                out=attn[h,
                                         qb * 512 + j * 128:qb * 512 + (j + 1) * 128,
                                         (g - 1) * 512:(g + 1) * 512],
                                in_=pn[j])
            # output projection partials for this q block
            for dc in range(4):
                ps_o = ppo.tile([128, 512], f32, tag="po", name="po")
                nc.tensor.matmul(ps_o, lhsT=wo_sbh[0][:, dc * 128:(dc + 1) * 128],
                                 rhs=ansT[0], start=True, stop=False)
                nc.tensor.matmul(ps_o, lhsT=wo_sbh[1][:, dc * 128:(dc + 1) * 128],
                                 rhs=ansT[1], start=False, stop=True)
                osb = outsb.tile([128, 512], f32, tag="osb")
                nc.scalar.copy(osb, ps_o)
                nc.sync.dma_start(out=outp[dc * 128:(dc + 1) * 128, qs], in_=osb)

    nc.finalize()
    return nc


def _get_program():
    global _PROGRAM
    if _PROGRAM is None:
        _PROGRAM = _build_program()
    return _PROGRAM


def _host_prep(query, key, value, mask_key, Wq, bq, Wk, bk, Wv, bv, Wo, bo):
    f = np.float32
    query, key, value = (np.asarray(a, f) for a in (query, key, value))
    Wq, bq, Wk, bk, Wv, bv, Wo, bo = (np.asarray(a, f)
                                      for a in (Wq, bq, Wk, bk, Wv, bv, Wo, bo))
    mask_key = np.asarray(mask_key)
    ones_row = np.ones((1, L), f)
    in_maps = []
    for c in range(8):
        b = c // 4
        heads = (2 * (c % 4), 2 * (c % 4) + 1)
        mrow = (np.float32(-1e18) * mask_key[b, 0].astype(f)).reshape(1, L)
        aux = np.concatenate([ones_row, mrow], axis=0)
        wpk = np.zeros((128, WP_COLS), f)
        for j, h in enumerate(heads):
            sl = slice(h * DH, (h + 1) * DH)
            for ci in range(4):
                rs = slice(ci * 128, (ci + 1) * 128)
                wpk[:, WQ_OFF + ci * 128 + j * DH:
                    WQ_OFF + ci * 128 + (j + 1) * DH] = Wq[sl, rs].T / 8.0
                wpk[:, WK_OFF + ci * 128 + j * DH:
                    WK_OFF + ci * 128 + (j + 1) * DH] = Wk[sl, rs].T
                wpk[:, WV_OFF + ci * 128 + j * DH:
                    WV_OFF + ci * 128 + (j + 1) * DH] = Wv[sl, rs].T
            wpk[j * DH:(j + 1) * DH, BQ_COL] = bq[sl] / 8.0
            wpk[j * DH:(j + 1) * DH, BK_COL] = bk[sl]
            wpk[0:64, WO_OFF + j * 512:WO_OFF + (j + 1) * 512] = Wo[:, sl].T
        wpk[:, ID_OFF:ID_OFF + 128] = np.eye(128, dtype=f)
        in_maps.append({
            "xq": np.ascontiguousarray(query[b].T),
            "xk": np.ascontiguousarray(key[b].T),
            "xv": np.ascontiguousarray(value[b].T),
            "wpk": wpk, "aux": aux,
        })
    return in_maps


def run(inputs, trace=False):
    """Returns ((out, attention), BassKernelResults)."""
    from concourse.bass_utils import run_bass_kernel_spmd

    nc = _get_program()
    in_maps = _host_prep(**inputs)
    res = run_bass_kernel_spmd(nc, in_maps, list(range(8)), trace=trace)

    Wo = np.asarray(inputs["Wo"], np.float32)
    bv = np.asarray(inputs["bv"], np.float32)
    bo = np.asarray(inputs["bo"], np.float32)

    attention = np.empty((B, H, L, L), np.float32)
    out = np.zeros((B, L, D), np.float32)
    for c, r in enumerate(res.results):
        b = c // 4
        h0 = 2 * (c % 4)
        attention[b, h0:h0 + 2] = r["attn"]
        out[b] += r["outp"].T
    out += (bv @ Wo.T + bo)[None, None, :]
    return (out, attention), res


def kernel(**inputs):
    result, _ = run(inputs, trace=False)
    return result


# revision 13
# speedup vs baseline: 1.3577x; 1.3577x over previous
"""Trainium2 Bass kernel for nn_MultiHeadAttention (B=2, L=4096, D=512, H=8).

Sharding: 16 (batch, head) attention slices over 8 NeuronCores — each core
handles one batch and two heads (core c -> batch c//4, heads 2*(c%4), 2*(c%4)+1).

Per-core device pipeline (all layouts chosen so the TensorE contraction dim is
on SBUF partitions and the big attention output DMAs are contiguous):
  - project qT/kT (head-dim on partitions, seq on free) and V (seq on
    partitions) from host-transposed inputs; bq/bk enter through an extra
    ones-row matmul term, the key-mask through an extra mask row appended to
    kT (dh 64->65) paired with a ones row on qT.
  - per 512-wide q block: S^T = kT.T @ qT per 128-wide k chunk (fp32r
    matmuls, two chunks per 2-bank PSUM tile), exp on ScalarE into SBUF, then
    answer_aug^T += V_aug.T @ E^T where V_aug carries a ones column so row 64
    of the accumulator is the softmax row sum.
  - reciprocals of the row sums (both orientations, via a PE transpose of the
    row-sum row and a GPSIMD partition-broadcast), normalize the answer,
    output projection partials out_pT = Wo_stack.T @ ansT into DRAM.
  - attention output: PE-transpose E^T tiles back to natural (q-partition)
    layout, fuse the softmax normalization into the PSUM->SBUF move
    (tensor_scalar with a per-partition reciprocal), pair two 512-wide strips
    and DMA via the otherwise-idle GPSIMD/SWDGE queue (4 KB lines).
Host: builds per-core transposed inputs and a single packed weight image,
sums the per-core output projection partials, and adds the bv/bo contribution
(exact because softmax rows sum to one).
"""

import os
import sys

import numpy as np

for _p in (
    "/root/.axon_site",
    "/root/.axon_site/_ro/trn_rl_repo",
    "/root/.axon_site/_ro/pypackages",
    "/opt/trn_rl_repo",
):
    if os.path.isdir(_p) and _p not in sys.path:
        sys.path.append(_p)

B, L, D, H, DH = 2, 4096, 512, 8, 64
NQB = 8      # q blocks of 512
NKC = 32     # k chunks of 128

# packed-weight column offsets: wq(5x128) wk(5x128) wv(4x128) ident wo0 wo1
WQ_OFF, WK_OFF, WV_OFF = 0, 640, 1280
ID_OFF, WO_OFF = 1792, 1920
BQ_COL, BK_COL = 2944, 2945
WP_COLS = 2946

_PROGRAM = None


def _build_program():
    import concourse.mybir as mybir
    import concourse.tile as tile
    from concourse import bacc

    dt = mybir.dt
    f32, f32r = dt.float32, dt.float32r
    bf16 = dt.bfloat16
    AF = mybir.ActivationFunctionType
    from contextlib import ExitStack

    nc = bacc.Bacc()
    xq = nc.declare_dram_parameter("xq", [D, L], f32r, isOutput=False).ap()
    xk = nc.declare_dram_parameter("xk", [D, L], f32r, isOutput=False).ap()
    xv = nc.declare_dram_parameter("xv", [D, L], f32r, isOutput=False).ap()
    wpk_d = nc.declare_dram_parameter("wpk", [128, WP_COLS], f32r,
                                      isOutput=False).ap()
    aux = nc.declare_dram_parameter("aux", [2, L], f32r, isOutput=False).ap()
    attn = nc.declare_dram_parameter("attn", [2, L, L], f32, isOutput=True).ap()
    outp = nc.declare_dram_parameter("outp", [D, L], f32, isOutput=True).ap()

    with ExitStack() as ctx:
        tc = ctx.enter_context(tile.TileContext(nc))
        persist = ctx.enter_context(tc.tile_pool(name="persist", bufs=1))
        small = ctx.enter_context(tc.tile_pool(name="small", bufs=2))
        ansp = ctx.enter_context(tc.tile_pool(name="ansp", bufs=2))
        outsb = ctx.enter_context(tc.tile_pool(name="outsb", bufs=2))
        ppo = ctx.enter_context(tc.tile_pool(name="ppo", bufs=1, space="PSUM"))

        wpk = persist.tile([128, WP_COLS], f32r, tag="wpk")
        nc.sync.dma_start(out=wpk, in_=wpk_d)
        identr = wpk[:, ID_OFF:ID_OFF + 128]
        wq_sb = [wpk[0:(128 if ci < 4 else 1),
                     WQ_OFF + ci * 128:WQ_OFF + ci * 128 + 128]
                 for ci in range(5)]
        wk_sb = [wpk[0:(128 if ci < 4 else 1),
                     WK_OFF + ci * 128:WK_OFF + ci * 128 + 128]
                 for ci in range(5)]
        wv_sb = [wpk[0:128, WV_OFF + ci * 128:WV_OFF + ci * 128 + 128]
                 for ci in range(4)]
        wo_sbh = [wpk[0:64, WO_OFF + h * 512:WO_OFF + (h + 1) * 512]
                  for h in range(2)]

        ident_bf = persist.tile([128, 128], bf16, tag="ident_bf")
        nc.scalar.copy(ident_bf, identr)
        rsf = persist.tile([128, 512], f32, tag="rsf")
        nc.vector.memset(rsf, 0.0)

        qT = [persist.tile([65, L], f32r, tag=f"qT{h}", name=f"qT{h}")
              for h in range(2)]
        kT = [persist.tile([65, L], f32r, tag=f"kT{h}", name=f"kT{h}")
              for h in range(2)]
        v2h = persist.tile([128, NKC, 130], bf16, tag="v2h")
        for h in range(2):
            nc.sync.dma_start(out=qT[h][64:65, :], in_=aux[0:1, :])
            nc.sync.dma_start(out=kT[h][64:65, :], in_=aux[1:2, :])
        nc.gpsimd.dma_start(out=v2h[:, :, 64:65], in_=aux[0:1, :].bitcast(f32))
        nc.gpsimd.dma_start(out=v2h[:, :, 129:130], in_=aux[0:1, :].bitcast(f32))

        # ---- projections (scoped pools; ci-outer accumulation into 8
        # psum banks so the 2 MB chunk loads double-buffer with compute) ----
        with tc.tile_pool(name="stage", bufs=4) as stage, \
             tc.tile_pool(name="pop", bufs=8, space="PSUM") as pop:
            for xap, wsb, dst in ((xq, wq_sb, qT), (xk, wk_sb, kT)):
                bc = BQ_COL if dst is qT else BK_COL
                bcol = wpk[:, bc:bc + 1]
                pss = [pop.tile([128, 512], f32, tag="pop", name="pop")
                       for _ in range(8)]
                for ci in range(4):
                    xt = stage.tile([128, L], f32r, tag="xs", name="xs")
                    nc.sync.dma_start(out=xt, in_=xap[ci * 128:(ci + 1) * 128, :])
                    for n in range(8):
                        nc.tensor.matmul(pss[n], lhsT=wsb[ci],
                                         rhs=xt[:, n * 512:(n + 1) * 512],
                                         start=(ci == 0), stop=(ci == 3))
                for n in range(8):
                    ns = slice(n * 512, (n + 1) * 512)
                    nc.scalar.add(dst[0][0:64, ns], pss[n][0:64, :],
                                  bcol[0:64, :].bitcast(f32))
                    nc.scalar.add(dst[1][0:64, ns], pss[n][64:128, :],
                                  bcol[64:128, :].bitcast(f32))

            xtv = []
            for ci in range(4):
                xt = stage.tile([128, L], f32r, tag="xs", name="xs")
                nc.sync.dma_start(out=xt, in_=xv[ci * 128:(ci + 1) * 128, :])
                xtv.append(xt)
            for n4 in range(8):
                psv = pop.tile([128, 512], f32, tag="pop", name="pop")
                for nn in range(4):
                    cs = slice(n4 * 512 + nn * 128, n4 * 512 + (nn + 1) * 128)
                    # strip-at-a-time: a start=True clears the whole bank's
                    # has_written bits, so each strip's 4-matmul group must
                    # finish before the next strip begins
                    for ci in range(4):
                        nc.tensor.matmul(psv[:, nn * 128:(nn + 1) * 128],
                                         lhsT=xtv[ci][:, cs], rhs=wv_sb[ci],
                                         start=(ci == 0), stop=(ci == 3))
                for nn in range(4):
                    kc = n4 * 4 + nn
                    nc.scalar.copy(v2h[:, kc, 0:64],
                                   psv[:, nn * 128:nn * 128 + 64])
                    nc.scalar.copy(v2h[:, kc, 65:129],
                                   psv[:, nn * 128 + 64:(nn + 1) * 128])

        # ---- main attention loop ----
        ppo = ctx.enter_context(tc.tile_pool(name="ppo", bufs=1, space="PSUM"))
        pps = ctx.enter_context(tc.tile_pool(name="pps", bufs=2, space="PSUM"))
        ppa = ctx.enter_context(tc.tile_pool(name="ppa", bufs=1, space="PSUM"))
        ppt = ctx.enter_context(tc.tile_pool(name="ppt", bufs=2, space="PSUM"))
        etp = ctx.enter_context(tc.tile_pool(name="etp", bufs=28))
        pnp = ctx.enter_context(tc.tile_pool(name="pnp", bufs=6))
        for qb in range(NQB):
            qs = slice(qb * 512, (qb + 1) * 512)
            ansT = [None, None]
            for h in range(2):
                ps_a = ppa.tile([128, 512], f32, tag="pa", name="pa")
                ets = []
                for kc2 in range(NKC // 2):
                    ps_s = pps.tile([128, 1024], f32, tag="ps", name="ps")
                    for hf in range(2):
                        kc = 2 * kc2 + hf
                        nc.tensor.matmul(ps_s[:, hf * 512:(hf + 1) * 512],
                                         lhsT=kT[h][:, kc * 128:(kc + 1) * 128],
                                         rhs=qT[h][:, qs], start=True, stop=True)
                    et = etp.tile([128, 1024], bf16, tag="et", name="et")
                    nc.scalar.activation(et, ps_s, AF.Exp)
                    for hf in range(2):
                        kc = 2 * kc2 + hf
                        nc.tensor.matmul(ps_a[0:65, :],
                                         lhsT=v2h[:, kc, 65 * h:65 * h + 65],
                                         rhs=et[:, hf * 512:(hf + 1) * 512],
                                         start=(kc == 0), stop=(kc == NKC - 1))
                        ets.append(et[:, hf * 512:(hf + 1) * 512])
                # row sums -> reciprocals in both orientations
                nc.scalar.copy(rsf[0:1, :], ps_a[64:65, :])
                ps_r = ppt.tile([128, 512], f32, tag="pt", name="pt")
                for j in range(4):
                    nc.tensor.transpose(ps_r[:, j * 128:(j + 1) * 128],
                                        rsf[:, j * 128:(j + 1) * 128],
                                        identr.bitcast(f32))
                recn = small.tile([128, 4], f32, tag="recn")
                nc.vector.reciprocal(
                    recn, ps_r.rearrange("p (j i) -> p j i", i=128)[:, :, 0])
                rr = small.tile([1, 512], f32, tag="rr")
                nc.vector.reciprocal(rr, rsf[0:1, :])
                rbc = small.tile([64, 512], f32, tag="rbc")
                nc.gpsimd.partition_broadcast(rbc, rr)
                at = ansp.tile([64, 512], f32r, tag=f"ans{h}", name=f"ans{h}")
                nc.vector.tensor_mul(at, ps_a[0:64, :], rbc.bitcast(f32r))
                ansT[h] = at
                # attention out: transpose + normalized PSUM->SBUF, DMA pairs
                pn = [None] * 4
                for g in range(8):
                    for j in range(4):
                        ps_t = ppt.tile([128, 512], bf16, tag="pt", name="pt")
                        for t in range(4):
                            nc.tensor.transpose(
                                ps_t[:, t * 128:(t + 1) * 128],
                                ets[g * 4 + t][:, j * 128:(j + 1) * 128],
                                ident_bf)
                        if g % 2 == 0:
                            pn[j] = pnp.tile([128, 1024], bf16, tag="pn",
                                             name="pn")
                        nc.vector.tensor_scalar_mul(
                            pn[j][:, (g % 2) * 512:(g % 2 + 1) * 512],
                            ps_t, recn[:, j:j + 1])
                        if g % 2 == 1:
                            nc.gpsimd.dma_start(
                                out=attn[h,
                                         qb * 512 + j * 128:qb * 512 + (j + 1) * 128,
                                         (g - 1) * 512:(g + 1) * 512],
                                in_=pn[j])
            # output projection partials for this q block
            for dc in range(4):
                ps_o = ppo.tile([128, 512], f32, tag="po", name="po")
                nc.tensor.matmul(ps_o, lhsT=wo_sbh[0][:, dc * 128:(dc + 1) * 128],
                                 rhs=ansT[0], start=True, stop=False)
                nc.tensor.matmul(ps_o, lhsT=wo_sbh[1][:, dc * 128:(dc + 1) * 128],
                                 rhs=ansT[1], start=False, stop=True)
                osb = outsb.tile([128, 512], f32, tag="osb")
                nc.scalar.copy(osb, ps_o)
                nc.sync.dma_start(out=outp[dc * 128:(dc + 1) * 128, qs], in_=osb)

    nc.finalize()
    return nc


def _get_program():
    global _PROGRAM
    if _PROGRAM is None:
        _PROGRAM = _build_program()
    return _PROGRAM


def _host_prep(query, key, value, mask_key, Wq, bq, Wk, bk, Wv, bv, Wo, bo):
    f = np.float32
    query, key, value = (np.asarray(a, f) for a in (query, key, value))
    Wq, bq, Wk, bk, Wv, bv, Wo, bo = (np.asarray(a, f)
                                      for a in (Wq, bq, Wk, bk, Wv, bv, Wo, bo))
    mask_key = np.asarray(mask_key)
    ones_row = np.ones((1, L), f)
    in_maps = []
    for c in range(8):
        b = c // 4
        heads = (2 * (c % 4), 2 * (c % 4) + 1)
        mrow = (np.float32(-1e18) * mask_key[b, 0].astype(f)).reshape(1, L)
        aux = np.concatenate([ones_row, mrow], axis=0)
        wpk = np.zeros((128, WP_COLS), f)
        for j, h in enumerate(heads):
            sl = slice(h * DH, (h + 1) * DH)
            for ci in range(4):
                rs = slice(ci * 128, (ci + 1) * 128)
                wpk[:, WQ_OFF + ci * 128 + j * DH:
                    WQ_OFF + ci * 128 + (j + 1) * DH] = Wq[sl, rs].T / 8.0
                wpk[:, WK_OFF + ci * 128 + j * DH:
                    WK_OFF + ci * 128 + (j + 1) * DH] = Wk[sl, rs].T
                wpk[:, WV_OFF + ci * 128 + j * DH:
                    WV_OFF + ci * 128 + (j + 1) * DH] = Wv[sl, rs].T
            wpk[j * DH:(j + 1) * DH, BQ_COL] = bq[sl] / 8.0
            wpk[j * DH:(j + 1) * DH, BK_COL] = bk[sl]
            wpk[0:64, WO_OFF + j * 512:WO_OFF + (j + 1) * 512] = Wo[:, sl].T
        wpk[:, ID_OFF:ID_OFF + 128] = np.eye(128, dtype=f)
        in_maps.append({
            "xq": np.ascontiguousarray(query[b].T),
            "xk": np.ascontiguousarray(key[b].T),
            "xv": np.ascontiguousarray(value[b].T),
            "wpk": wpk, "aux": aux,
        })
    return in_maps


def run(inputs, trace=False):
    """Returns ((out, attention), BassKernelResults)."""
    from concourse.bass_utils import run_bass_kernel_spmd

    nc = _get_program()
    in_maps = _host_prep(**inputs)
    res = run_bass_kernel_spmd(nc, in_maps, list(range(8)), trace=trace)

    Wo = np.asarray(inputs["Wo"], np.float32)
    bv = np.asarray(inputs["bv"], np.float32)
    bo = np.asarray(inputs["bo"], np.float32)

    attention = np.empty((B, H, L, L), np.float32)
    out = np.zeros((B, L, D), np.float32)
    for c, r in enumerate(res.results):
        b = c // 4
        h0 = 2 * (c % 4)
        attention[b, h0:h0 + 2] = r["attn"]
        out[b] += r["outp"].T
    out += (bv @ Wo.T + bo)[None, None, :]
    return (out, attention), res


def kernel(**inputs):
    result, _ = run(inputs, trace=False)
    return result


# revision 17
# speedup vs baseline: 1.5499x; 1.1415x over previous
"""Trainium2 Bass kernel for nn_MultiHeadAttention (B=2, L=4096, D=512, H=8).

Sharding: 16 (batch, head) attention slices over 8 NeuronCores — each core
handles one batch and two heads (core c -> batch c//4, heads 2*(c%4), 2*(c%4)+1).

Per-core device pipeline (all layouts chosen so the TensorE contraction dim is
on SBUF partitions and the big attention output DMAs are contiguous):
  - project qT/kT (head-dim on partitions, seq on free) and V (seq on
    partitions) from host-transposed inputs; bq/bk enter through an extra
    ones-row matmul term, the key-mask through an extra mask row appended to
    kT (dh 64->65) paired with a ones row on qT.
  - per 512-wide q block: S^T = kT.T @ qT per 128-wide k chunk (fp32r
    matmuls, two chunks per 2-bank PSUM tile), exp on ScalarE into SBUF, then
    answer_aug^T += V_aug.T @ E^T where V_aug carries a ones column so row 64
    of the accumulator is the softmax row sum.
  - reciprocals of the row sums (both orientations, via a PE transpose of the
    row-sum row and a GPSIMD partition-broadcast), normalize the answer,
    output projection partials out_pT = Wo_stack.T @ ansT into DRAM.
  - attention output: PE-transpose E^T tiles back to natural (q-partition)
    layout, fuse the softmax normalization into the PSUM->SBUF move
    (tensor_scalar with a per-partition reciprocal), pair two 512-wide strips
    and DMA via the otherwise-idle GPSIMD/SWDGE queue (4 KB lines).
Host: builds per-core transposed inputs and a single packed weight image,
sums the per-core output projection partials, and adds the bv/bo contribution
(exact because softmax rows sum to one).
"""

import os
import sys

import numpy as np

for _p in (
    "/root/.axon_site",
    "/root/.axon_site/_ro/trn_rl_repo",
    "/root/.axon_site/_ro/pypackages",
    "/opt/trn_rl_repo",
):
    if os.path.isdir(_p) and _p not in sys.path:
        sys.path.append(_p)

B, L, D, H, DH = 2, 4096, 512, 8, 64
NQB = 8      # q blocks of 512
NKC = 32     # k chunks of 128

# packed-weight column offsets: wq(5x128) wk(5x128) wv(4x128) ident wo0 wo1
WQ_OFF, WK_OFF, WV_OFF = 0, 640, 1280
ID_OFF, WO_OFF = 1792, 1920
BQ_COL, BK_COL = 2944, 2945
WP_COLS = 2946

_PROGRAM = None


def _build_program():
    import concourse.mybir as mybir
    import concourse.tile as tile
    from concourse import bacc

    dt = mybir.dt
    f32, f32r = dt.float32, dt.float32r
    bf16 = dt.bfloat16
    AF = mybir.ActivationFunctionType
    from contextlib import ExitStack

    nc = bacc.Bacc()
    xq = nc.declare_dram_parameter("xq", [D, L], f32r, isOutput=False).ap()
    xk = nc.declare_dram_parameter("xk", [D, L], f32r, isOutput=False).ap()
    xv = nc.declare_dram_parameter("xv", [D, L], f32r, isOutput=False).ap()
    wpk_d = nc.declare_dram_parameter("wpk", [128, WP_COLS], f32r,
                                      isOutput=False).ap()
    aux = nc.declare_dram_parameter("aux", [2, L], f32r, isOutput=False).ap()
    attn = nc.declare_dram_parameter("attn", [2, L, L], f32, isOutput=True).ap()
    outp = nc.declare_dram_parameter("outp", [D, L], f32, isOutput=True).ap()

    with ExitStack() as ctx:
        tc = ctx.enter_context(tile.TileContext(nc))
        persist = ctx.enter_context(tc.tile_pool(name="persist", bufs=1))
        small = ctx.enter_context(tc.tile_pool(name="small", bufs=2))
        ansp = ctx.enter_context(tc.tile_pool(name="ansp", bufs=2))
        outsb = ctx.enter_context(tc.tile_pool(name="outsb", bufs=2))
        ppo = ctx.enter_context(tc.tile_pool(name="ppo", bufs=1, space="PSUM"))

        wpk = persist.tile([128, WP_COLS], f32r, tag="wpk")
        nc.sync.dma_start(out=wpk, in_=wpk_d)
        identr = wpk[:, ID_OFF:ID_OFF + 128]
        wq_sb = [wpk[0:(128 if ci < 4 else 1),
                     WQ_OFF + ci * 128:WQ_OFF + ci * 128 + 128]
                 for ci in range(5)]
        wk_sb = [wpk[0:(128 if ci < 4 else 1),
                     WK_OFF + ci * 128:WK_OFF + ci * 128 + 128]
                 for ci in range(5)]
        wv_sb = [wpk[0:128, WV_OFF + ci * 128:WV_OFF + ci * 128 + 128]
                 for ci in range(4)]
        wo_sbh = [wpk[0:64, WO_OFF + h * 512:WO_OFF + (h + 1) * 512]
                  for h in range(2)]

        ident_bf = persist.tile([128, 128], bf16, tag="ident_bf")
        nc.scalar.copy(ident_bf, identr)
        rsf = persist.tile([128, 512], f32, tag="rsf")
        nc.vector.memset(rsf, 0.0)

        qT = [persist.tile([65, L], f32r, tag=f"qT{h}", name=f"qT{h}")
              for h in range(2)]
        kT = [persist.tile([65, L], f32r, tag=f"kT{h}", name=f"kT{h}")
              for h in range(2)]
        v2h = persist.tile([128, NKC, 130], bf16, tag="v2h")
        for h in range(2):
            nc.sync.dma_start(out=qT[h][64:65, :], in_=aux[0:1, :])
            nc.sync.dma_start(out=kT[h][64:65, :], in_=aux[1:2, :])
        nc.gpsimd.dma_start(out=v2h[:, :, 64:65], in_=aux[0:1, :].bitcast(f32))
        nc.gpsimd.dma_start(out=v2h[:, :, 129:130], in_=aux[0:1, :].bitcast(f32))

        # ---- projections (scoped pools; ci-outer accumulation into 8
        # psum banks so the 2 MB chunk loads double-buffer with compute) ----
        with tc.tile_pool(name="stage", bufs=4) as stage, \
             tc.tile_pool(name="pop", bufs=8, space="PSUM") as pop:
            for xap, wsb, dst in ((xq, wq_sb, qT), (xk, wk_sb, kT)):
                bc = BQ_COL if dst is qT else BK_COL
                bcol = wpk[:, bc:bc + 1]
                pss = [pop.tile([128, 512], f32, tag="pop", name="pop")
                       for _ in range(8)]
                for ci in range(4):
                    xt = stage.tile([128, L], f32r, tag="xs", name="xs")
                    nc.sync.dma_start(out=xt, in_=xap[ci * 128:(ci + 1) * 128, :])
                    for n in range(8):
                        nc.tensor.matmul(pss[n], lhsT=wsb[ci],
                                         rhs=xt[:, n * 512:(n + 1) * 512],
                                         start=(ci == 0), stop=(ci == 3))
                for n in range(8):
                    ns = slice(n * 512, (n + 1) * 512)
                    nc.scalar.add(dst[0][0:64, ns], pss[n][0:64, :],
                                  bcol[0:64, :].bitcast(f32))
                    nc.scalar.add(dst[1][0:64, ns], pss[n][64:128, :],
                                  bcol[64:128, :].bitcast(f32))

            xtv = []
            for ci in range(4):
                xt = stage.tile([128, L], f32r, tag="xs", name="xs")
                nc.sync.dma_start(out=xt, in_=xv[ci * 128:(ci + 1) * 128, :])
                xtv.append(xt)
            for n4 in range(8):
                psv = pop.tile([128, 512], f32, tag="pop", name="pop")
                for nn in range(4):
                    cs = slice(n4 * 512 + nn * 128, n4 * 512 + (nn + 1) * 128)
                    # strip-at-a-time: a start=True clears the whole bank's
                    # has_written bits, so each strip's 4-matmul group must
                    # finish before the next strip begins
                    for ci in range(4):
                        nc.tensor.matmul(psv[:, nn * 128:(nn + 1) * 128],
                                         lhsT=xtv[ci][:, cs], rhs=wv_sb[ci],
                                         start=(ci == 0), stop=(ci == 3))
                for nn in range(4):
                    kc = n4 * 4 + nn
                    nc.scalar.copy(v2h[:, kc, 0:64],
                                   psv[:, nn * 128:nn * 128 + 64])
                    nc.scalar.copy(v2h[:, kc, 65:129],
                                   psv[:, nn * 128 + 64:(nn + 1) * 128])

        # ---- main attention loop ----
        ppo = ctx.enter_context(tc.tile_pool(name="ppo", bufs=1, space="PSUM"))
        pps = ctx.enter_context(tc.tile_pool(name="pps", bufs=2, space="PSUM"))
        ppa = ctx.enter_context(tc.tile_pool(name="ppa", bufs=1, space="PSUM"))
        ppt = ctx.enter_context(tc.tile_pool(name="ppt", bufs=2, space="PSUM"))
        etp = ctx.enter_context(tc.tile_pool(name="etp", bufs=28))
        pnp = ctx.enter_context(tc.tile_pool(name="pnp", bufs=6))
        for qb in range(NQB):
            qs = slice(qb * 512, (qb + 1) * 512)
            ansT = [None, None]
            for h in range(2):
                ps_a = ppa.tile([128, 512], f32, tag="pa", name="pa")
                ets = []
                for kc2 in range(NKC // 2):
                    ps_s = pps.tile([128, 1024], f32, tag="ps", name="ps")
                    for hf in range(2):
                        kc = 2 * kc2 + hf
                        nc.tensor.matmul(ps_s[:, hf * 512:(hf + 1) * 512],
                                         lhsT=kT[h][:, kc * 128:(kc + 1) * 128],
                                         rhs=qT[h][:, qs], start=True, stop=True)
                    et = etp.tile([128, 1024], bf16, tag="et", name="et")
                    nc.scalar.activation(et, ps_s, AF.Exp)
                    for hf in range(2):
                        kc = 2 * kc2 + hf
                        nc.tensor.matmul(ps_a[0:65, :],
                                         lhsT=v2h[:, kc, 65 * h:65 * h + 65],
                                         rhs=et[:, hf * 512:(hf + 1) * 512],
                                         start=(kc == 0), stop=(kc == NKC - 1))
                        ets.append(et[:, hf * 512:(hf + 1) * 512])
                # row sums -> reciprocals in both orientations
                nc.scalar.copy(rsf[0:1, :], ps_a[64:65, :])
                ps_r = ppt.tile([128, 512], f32, tag="pt", name="pt")
                for j in range(4):
                    nc.tensor.transpose(ps_r[:, j * 128:(j + 1) * 128],
                                        rsf[:, j * 128:(j + 1) * 128],
                                        identr.bitcast(f32))
                recn = small.tile([128, 4], f32, tag="recn")
                nc.vector.reciprocal(
                    recn, ps_r.rearrange("p (j i) -> p j i", i=128)[:, :, 0])
                rr = small.tile([1, 512], f32, tag="rr")
                nc.vector.reciprocal(rr, rsf[0:1, :])
                rbc = small.tile([64, 512], f32, tag="rbc")
                nc.gpsimd.partition_broadcast(rbc, rr)
                at = ansp.tile([64, 512], f32r, tag=f"ans{h}", name=f"ans{h}")
                nc.vector.tensor_mul(at, ps_a[0:64, :], rbc.bitcast(f32r))
                ansT[h] = at
                # attention out: transpose + normalized PSUM->SBUF, DMA pairs
                pn = [None] * 4
                for g in range(8):
                    for j in range(4):
                        ps_t = ppt.tile([128, 512], bf16, tag="pt", name="pt")
                        for t in range(4):
                            nc.tensor.transpose(
                                ps_t[:, t * 128:(t + 1) * 128],
                                ets[g * 4 + t][:, j * 128:(j + 1) * 128],
                                ident_bf)
                        if g % 2 == 0:
                            pn[j] = pnp.tile([128, 1024], bf16, tag="pn",
                                             name="pn")
                        nc.vector.tensor_scalar_mul(
                            pn[j][:, (g % 2) * 512:(g % 2 + 1) * 512],
                            ps_t, recn[:, j:j + 1])
                        if g % 2 == 1:
                            nc.gpsimd.dma_start(
                                out=attn[h,
                                         qb * 512 + j * 128:qb * 512 + (j + 1) * 128,
                                         (g - 1) * 512:(g + 1) * 512],
                                in_=pn[j])
            # output projection partials for this q block
            for dc in range(4):
                ps_o = ppo.tile([128, 512], f32, tag="po", name="po")
                nc.tensor.matmul(ps_o, lhsT=wo_sbh[0][:, dc * 128:(dc + 1) * 128],
                                 rhs=ansT[0], start=True, stop=False)
                nc.tensor.matmul(ps_o, lhsT=wo_sbh[1][:, dc * 128:(dc + 1) * 128],
                                 rhs=ansT[1], start=False, stop=True)
                osb = outsb.tile([128, 512], f32, tag="osb")
                nc.scalar.copy(osb, ps_o)
                nc.sync.dma_start(out=outp[dc * 128:(dc + 1) * 128, qs], in_=osb)

    nc.finalize()
    return nc


def _get_program():
    global _PROGRAM
    if _PROGRAM is None:
        _PROGRAM = _build_program()
    return _PROGRAM


def _host_prep(query, key, value, mask_key, Wq, bq, Wk, bk, Wv, bv, Wo, bo):
    f = np.float32
    query, key, value = (np.asarray(a, f) for a in (query, key, value))
    Wq, bq, Wk, bk, Wv, bv, Wo, bo = (np.asarray(a, f)
                                      for a in (Wq, bq, Wk, bk, Wv, bv, Wo, bo))
    mask_key = np.asarray(mask_key)
    ones_row = np.ones((1, L), f)
    in_maps = []
    for c in range(8):
        b = c // 4
        heads = (2 * (c % 4), 2 * (c % 4) + 1)
        mrow = (np.float32(-1e18) * mask_key[b, 0].astype(f)).reshape(1, L)
        aux = np.concatenate([ones_row, mrow], axis=0)
        wpk = np.zeros((128, WP_COLS), f)
        for j, h in enumerate(heads):
            sl = slice(h * DH, (h + 1) * DH)
            for ci in range(4):
                rs = slice(ci * 128, (ci + 1) * 128)
                wpk[:, WQ_OFF + ci * 128 + j * DH:
                    WQ_OFF + ci * 128 + (j + 1) * DH] = Wq[sl, rs].T / 8.0
                wpk[:, WK_OFF + ci * 128 + j * DH:
                    WK_OFF + ci * 128 + (j + 1) * DH] = Wk[sl, rs].T
                wpk[:, WV_OFF + ci * 128 + j * DH:
                    WV_OFF + ci * 128 + (j + 1) * DH] = Wv[sl, rs].T
            wpk[j * DH:(j + 1) * DH, BQ_COL] = bq[sl] / 8.0
            wpk[j * DH:(j + 1) * DH, BK_COL] = bk[sl]
            wpk[0:64, WO_OFF + j * 512:WO_OFF + (j + 1) * 512] = Wo[:, sl].T
        wpk[:, ID_OFF:ID_OFF + 128] = np.eye(128, dtype=f)
        in_maps.append({
            "xq": np.ascontiguousarray(query[b].T),
            "xk": np.ascontiguousarray(key[b].T),
            "xv": np.ascontiguousarray(value[b].T),
            "wpk": wpk, "aux": aux,
        })
    return in_maps


def run(inputs, trace=False):
    """Returns ((out, attention), BassKernelResults)."""
    from concourse.bass_utils import run_bass_kernel_spmd

    nc = _get_program()
    in_maps = _host_prep(**inputs)
    res = run_bass_kernel_spmd(nc, in_maps, list(range(8)), trace=trace)

    Wo = np.asarray(inputs["Wo"], np.float32)
    bv = np.asarray(inputs["bv"], np.float32)
    bo = np.asarray(inputs["bo"], np.float32)

    attention = np.empty((B, H, L, L), np.float32)
    out = np.zeros((B, L, D), np.float32)
    for c, r in enumerate(res.results):
        b = c // 4
        h0 = 2 * (c % 4)
        attention[b, h0:h0 + 2] = r["attn"]
        out[b] += r["outp"].T
    out += (bv @ Wo.T + bo)[None, None, :]
    return (out, attention), res


def kernel(**inputs):
    result, _ = run(inputs, trace=False)
    return result
